# revision 3
# baseline (speedup 1.0000x reference)
"""Coupled-map-lattice kernel for Trainium2, data-parallel over 8 NeuronCores.

Reference recurrence (per row n, channels c=0..255, 20 steps):
    mapped = r * g * (1 - g)
    local  = circular 5-tap conv of mapped over c
    glob   = mapped @ W_cc
    g'     = (1-beta)*((1-eps)*mapped + eps*0.5*(local+glob)) + beta*drive
    out    = clip(g_20, 1e-4, 1-1e-4)

Folded form used on device (host precomputes A_neg, qc):
    mapped = r*(1/4 - t),  t = (g - 1/2)^2
    g'     = t @ A_neg + bias2,   bias2 = qc + beta*drive   (constant over steps)
where A[c',c] = (1-beta_c)*[(1-eps_c)*I + eps_c*0.5*(B + W_cc)][c',c],
      B the circulant 5-tap matrix, A_neg = -(r ⊙rows A), qc = 1/4 * (r @ A).

Per-core loop (state kept transposed: channels on partitions):
    ACT: t = Square(g - 0.5)      (rounded to float32r)
    PE : psum = t @ A_neg         (fp32r matmuls, K=256 as 2x128)
    DVE: g' = psum + bias2
"""

import numpy as np

N, C, KTAPS, STEPS = 131072, 256, 5, 20
N_CORES = 8
N_SHARD = N // N_CORES          # 16384 rows per core
CHUNK = 2048                    # rows resident on-chip per chunk
PSUM_W = 512                    # matmul moving free dim / psum bank width
PSUM_TILE_W = 2048              # psum tile width (4 banks) for wide DVE reads

_CACHED_NC = None


def _build_nc():
    import concourse.tile as tile
    from concourse import bacc, mybir

    f32 = mybir.dt.float32
    f32r = mybir.dt.float32r
    Act = mybir.ActivationFunctionType
    Alu = mybir.AluOpType

    nc = bacc.Bacc("TRN2", target_bir_lowering=False)
    driveT = nc.declare_dram_parameter("driveT", [C, N_SHARD], f32, isOutput=False)
    a_blk = nc.declare_dram_parameter("a_blk", [128, 512], f32, isOutput=False)
    vecs = nc.declare_dram_parameter("vecs", [128, 4], f32, isOutput=False)
    outT = nc.declare_dram_parameter("outT", [C, N_SHARD], f32, isOutput=True)

    n_chunks = N_SHARD // CHUNK
    n_ptiles = CHUNK // PSUM_TILE_W      # psum tiles per (chunk, ctile)
    n_sub = PSUM_TILE_W // PSUM_W        # matmul subtiles per psum tile

    with tile.TileContext(nc) as tc:
        with (
            tc.tile_pool(name="const", bufs=1) as constp,
            tc.tile_pool(name="io", bufs=2) as iop,
            tc.tile_pool(name="state", bufs=2) as statep,
            tc.tile_pool(name="psum", bufs=2, space="PSUM") as psump,
        ):
            # ---- constants ----
            a_raw = constp.tile([128, 512], f32)
            nc.gpsimd.dma_start(a_raw[:], a_blk[:])
            a_t = constp.tile([128, 512], f32r)
            nc.scalar.copy(a_t[:], a_raw[:])          # round weights to fp32r
            v = constp.tile([128, 4], f32)
            nc.gpsimd.dma_start(v[:], vecs[:])
            negh = constp.tile([128, 1], f32)
            nc.vector.memset(negh[:], -0.5)

            for ci in range(n_chunks):
                col0 = ci * CHUNK
                # drive chunk in (also doubles as the output staging buffer)
                d = [iop.tile([128, CHUNK], f32, tag=f"d{j}", name=f"d{j}_{ci}") for j in range(2)]
                for j in range(2):
                    nc.gpsimd.dma_start(
                        d[j][:], driveT[j * 128:(j + 1) * 128, col0:col0 + CHUNK]
                    )
                tA = [statep.tile([128, CHUNK], f32r, tag=f"tA{j}", name=f"tA{j}_{ci}") for j in range(2)]
                tB = [statep.tile([128, CHUNK], f32r, tag=f"tB{j}", name=f"tB{j}_{ci}") for j in range(2)]
                bias = [statep.tile([128, CHUNK], f32, tag=f"b{j}", name=f"b{j}_{ci}") for j in range(2)]
                g = [statep.tile([128, CHUNK], f32, tag=f"g{j}", name=f"g{j}_{ci}") for j in range(2)]
                ob = [iop.tile([128, CHUNK], f32, tag=f"o{j}", name=f"o{j}_{ci}") for j in range(2)]

                # t0 = Square(drive - 0.5) ; bias2 = beta*drive + qc
                for j in range(2):
                    nc.scalar.activation(tA[j][:], d[j][:], Act.Square,
                                         bias=negh[:], scale=1.0)
                for j in range(2):
                    nc.vector.tensor_scalar(
                        bias[j][:], d[j][:], v[:, j:j + 1], v[:, 2 + j:3 + j],
                        Alu.mult, Alu.add,
                    )

                cur, nxt = tA, tB
                for step in range(STEPS):
                    last = step == STEPS - 1
                    for j in range(2):
                        for p in range(n_ptiles):
                            ps = psump.tile([128, PSUM_TILE_W], f32, tag="ps", name=f"ps_{ci}_{step}_{j}_{p}")
                            pc0 = p * PSUM_TILE_W
                            for s in range(n_sub):
                                sl_p = slice(s * PSUM_W, (s + 1) * PSUM_W)
                                sl_c = slice(pc0 + s * PSUM_W, pc0 + (s + 1) * PSUM_W)
                                nc.tensor.matmul(
                                    ps[:, sl_p], a_t[:, (0 * 2 + j) * 128:(0 * 2 + j) * 128 + 128],
                                    cur[0][:, sl_c], start=True, stop=False,
                                )
                                nc.tensor.matmul(
                                    ps[:, sl_p], a_t[:, (1 * 2 + j) * 128:(1 * 2 + j) * 128 + 128],
                                    cur[1][:, sl_c], start=False, stop=True,
                                )
                            sl_t = slice(pc0, pc0 + PSUM_TILE_W)
                            if not last:
                                nc.vector.tensor_tensor(
                                    g[j][:, sl_t], ps[:], bias[j][:, sl_t], Alu.add
                                )
                                nc.scalar.activation(
                                    nxt[j][:, sl_t], g[j][:, sl_t], Act.Square,
                                    bias=negh[:], scale=1.0,
                                )
                            else:
                                nc.vector.tensor_tensor(
                                    g[j][:, sl_t], ps[:], bias[j][:, sl_t], Alu.add
                                )
                                nc.vector.tensor_scalar(
                                    ob[j][:, sl_t], g[j][:, sl_t],
                                    1e-4, 1.0 - 1e-4, Alu.max, Alu.min,
                                )
                    cur, nxt = nxt, cur

                for j in range(2):
                    nc.gpsimd.dma_start(
                        outT[j * 128:(j + 1) * 128, col0:col0 + CHUNK], ob[j][:]
                    )
    nc.compile()
    return nc


def _get_nc():
    global _CACHED_NC
    if _CACHED_NC is None:
        _CACHED_NC = _build_nc()
    return _CACHED_NC


def _fold_constants(r, eps, beta, K_local, W_cc):
    """Host-side fold of the per-step linear operator into A_neg / qc."""
    pad = KTAPS // 2
    cp = np.arange(C)[:, None]
    c = np.arange(C)[None, :]
    j = (cp - c + pad) % C
    B = np.where(j < KTAPS, K_local.astype(np.float64)[np.minimum(j, KTAPS - 1)], 0.0)
    A = (1.0 - beta.astype(np.float64))[None, :] * (
        (1.0 - eps.astype(np.float64))[None, :] * np.eye(C)
        + eps.astype(np.float64)[None, :] * 0.5 * (B + W_cc.astype(np.float64))
    )
    A_r = r.astype(np.float64)[:, None] * A
    A_neg = (-A_r).astype(np.float32)          # [C, C]; g' = t @ A_neg + bias2
    qc = (0.25 * A_r.sum(axis=0)).astype(np.float32)   # [C]
    return A_neg, qc


def _pack_inputs(drive, r, eps, beta, K_local, W_cc):
    A_neg, qc = _fold_constants(r, eps, beta, K_local, W_cc)
    # lhsT blocks: block (k, m) at columns (k*2+m)*128
    a_blk = np.concatenate(
        [A_neg[k * 128:(k + 1) * 128, m * 128:(m + 1) * 128]
         for k in range(2) for m in range(2)], axis=1
    ).astype(np.float32)                       # [128, 512]
    vecs = np.stack(
        [beta[0:128], beta[128:256], qc[0:128], qc[128:256]], axis=1
    ).astype(np.float32)                       # [128, 4]
    driveT = np.ascontiguousarray(drive.T.astype(np.float32))   # [C, N]
    in_maps = []
    for i in range(N_CORES):
        shard = np.ascontiguousarray(driveT[:, i * N_SHARD:(i + 1) * N_SHARD])
        in_maps.append({"driveT": shard, "a_blk": a_blk, "vecs": vecs})
    return in_maps


def run(drive, r, eps, beta, K_local, W_cc, trace=False, trace_kwargs=None):
    from concourse.bass_utils import run_bass_kernel_spmd

    nc = _get_nc()
    in_maps = _pack_inputs(drive, r, eps, beta, K_local, W_cc)
    res = run_bass_kernel_spmd(
        nc, in_maps, core_ids=list(range(N_CORES)),
        trace=trace, **(trace_kwargs or {}),
    )
    outT = np.concatenate([res.results[i]["outT"] for i in range(N_CORES)], axis=1)
    out = np.ascontiguousarray(outT.T).astype(np.float32)
    return out, res


def kernel(drive, r, eps, beta, K_local, W_cc):
    out, _ = run(
        np.asarray(drive), np.asarray(r), np.asarray(eps), np.asarray(beta),
        np.asarray(K_local), np.asarray(W_cc),
    )
    return out


# revision 4
# speedup vs baseline: 1.8381x; 1.8381x over previous
"""Coupled-map-lattice kernel for Trainium2, data-parallel over 8 NeuronCores.

Reference recurrence (per row n, channels c=0..255, 20 steps):
    mapped = r * g * (1 - g)
    local  = circular 5-tap conv of mapped over c
    glob   = mapped @ W_cc
    g'     = (1-beta)*((1-eps)*mapped + eps*0.5*(local+glob)) + beta*drive
    out    = clip(g_20, 1e-4, 1-1e-4)

Folded form used on device (host precomputes A_neg, qc):
    mapped = r*(1/4 - t),  t = (g - 1/2)^2
    g'     = t @ A_neg + bias2,   bias2 = qc + beta*drive   (constant over steps)
where A[c',c] = (1-beta_c)*[(1-eps_c)*I + eps_c*0.5*(B + W_cc)][c',c],
      B the circulant 5-tap matrix, A_neg = -(r ⊙rows A), qc = 1/4 * (r @ A).

Per-core loop (state transposed: channels on partitions, fp16 matmul operands):
    ACT: t = Square(g - 0.5)             (rounded to fp16)
    PE : psum = t @ A_neg [+ I @ bias16]  (fp16 matmuls, K=256 as 2x128)
    DVE: g' = psum + bias2                (only for the non-offloaded half)
The identity-matmul offload moves half of the bias adds from DVE to PE.
"""

import numpy as np

N, C, KTAPS, STEPS = 131072, 256, 5, 20
N_CORES = 8
N_SHARD = N // N_CORES          # 16384 rows per core
CHUNK = 2048                    # rows resident on-chip per chunk
PSUM_W = 512                    # matmul moving free dim / psum bank width
PSUM_TILE_W = 1024              # psum tile width (2 banks)
OFF_W = 1024                    # leading columns per chunk offloaded to PE-identity

_CACHED_NC = None


def _build_nc():
    import concourse.tile as tile
    from concourse import bacc, mybir

    f32 = mybir.dt.float32
    f16 = mybir.dt.float16
    Act = mybir.ActivationFunctionType
    Alu = mybir.AluOpType

    nc = bacc.Bacc("TRN2", target_bir_lowering=False)
    driveT = nc.declare_dram_parameter("driveT", [C, N_SHARD], f32, isOutput=False)
    a_blk = nc.declare_dram_parameter("a_blk", [128, 640], f32, isOutput=False)
    vecs = nc.declare_dram_parameter("vecs", [128, 4], f32, isOutput=False)
    outT = nc.declare_dram_parameter("outT", [C, N_SHARD], f32, isOutput=True)

    n_chunks = N_SHARD // CHUNK
    n_ptiles = CHUNK // PSUM_TILE_W
    n_sub = PSUM_TILE_W // PSUM_W

    with tile.TileContext(nc) as tc:
        with (
            tc.tile_pool(name="const", bufs=1) as constp,
            tc.tile_pool(name="io", bufs=2) as iop,
            tc.tile_pool(name="state", bufs=2) as statep,
            tc.tile_pool(name="psum", bufs=4, space="PSUM") as psump,
        ):
            # ---- constants: A blocks (cols 0-511) + I (cols 512-639), fp16 ----
            a_raw = constp.tile([128, 640], f32)
            nc.gpsimd.dma_start(a_raw[:], a_blk[:])
            a_t = constp.tile([128, 640], f16)
            nc.scalar.copy(a_t[:], a_raw[:])
            v = constp.tile([128, 4], f32)
            nc.gpsimd.dma_start(v[:], vecs[:])
            negh = constp.tile([128, 1], f32)
            nc.vector.memset(negh[:], -0.5)

            for ci in range(n_chunks):
                col0 = ci * CHUNK
                d = [iop.tile([128, CHUNK], f32, tag=f"d{j}", name=f"d{j}_{ci}")
                     for j in range(2)]
                for j in range(2):
                    nc.gpsimd.dma_start(
                        d[j][:], driveT[j * 128:(j + 1) * 128, col0:col0 + CHUNK]
                    )
                tA = [statep.tile([128, CHUNK], f16, tag=f"tA{j}", name=f"tA{j}_{ci}")
                      for j in range(2)]
                tB = [statep.tile([128, CHUNK], f16, tag=f"tB{j}", name=f"tB{j}_{ci}")
                      for j in range(2)]
                b16 = [statep.tile([128, OFF_W], f16, tag=f"b16{j}", name=f"b16{j}_{ci}")
                       for j in range(2)]
                b32 = [statep.tile([128, CHUNK - OFF_W], f32, tag=f"b32{j}",
                                   name=f"b32{j}_{ci}") for j in range(2)]
                g = [statep.tile([128, CHUNK - OFF_W], f32, tag=f"g{j}",
                                 name=f"g{j}_{ci}") for j in range(2)]
                ob = [iop.tile([128, CHUNK], f32, tag=f"o{j}", name=f"o{j}_{ci}")
                      for j in range(2)]

                # t0 = Square(drive - 0.5); bias halves (fp16 for PE, fp32 for DVE)
                for j in range(2):
                    nc.scalar.activation(tA[j][:], d[j][:], Act.Square,
                                         bias=negh[:], scale=1.0)
                for j in range(2):
                    nc.vector.tensor_scalar(
                        b16[j][:], d[j][:, 0:OFF_W], v[:, j:j + 1], v[:, 2 + j:3 + j],
                        Alu.mult, Alu.add,
                    )
                    nc.vector.tensor_scalar(
                        b32[j][:], d[j][:, OFF_W:CHUNK], v[:, j:j + 1],
                        v[:, 2 + j:3 + j], Alu.mult, Alu.add,
                    )

                cur, nxt = tA, tB
                for step in range(STEPS):
                    last = step == STEPS - 1
                    for j in range(2):
                        for p in range(n_ptiles):
                            pc0 = p * PSUM_TILE_W
                            offl = pc0 < OFF_W
                            ps = psump.tile([128, PSUM_TILE_W], f32, tag="ps",
                                            name=f"ps_{ci}_{step}_{j}_{p}")
                            for s in range(n_sub):
                                sl_p = slice(s * PSUM_W, (s + 1) * PSUM_W)
                                c0 = pc0 + s * PSUM_W
                                sl_c = slice(c0, c0 + PSUM_W)
                                nc.tensor.matmul(
                                    ps[:, sl_p], a_t[:, (0 * 2 + j) * 128:(0 * 2 + j) * 128 + 128],
                                    cur[0][:, sl_c], start=True, stop=False,
                                )
                                nc.tensor.matmul(
                                    ps[:, sl_p], a_t[:, (1 * 2 + j) * 128:(1 * 2 + j) * 128 + 128],
                                    cur[1][:, sl_c], start=False, stop=not offl,
                                )
                                if offl:
                                    nc.tensor.matmul(
                                        ps[:, sl_p], a_t[:, 512:640],
                                        b16[j][:, sl_c], start=False, stop=True,
                                    )
                            sl_t = slice(pc0, pc0 + PSUM_TILE_W)
                            if offl:
                                if not last:
                                    nc.scalar.activation(
                                        nxt[j][:, sl_t], ps[:], Act.Square,
                                        bias=negh[:], scale=1.0,
                                    )
                                else:
                                    nc.vector.tensor_scalar(
                                        ob[j][:, sl_t], ps[:],
                                        1e-4, 1.0 - 1e-4, Alu.max, Alu.min,
                                    )
                            else:
                                sl_g = slice(pc0 - OFF_W, pc0 - OFF_W + PSUM_TILE_W)
                                nc.vector.tensor_tensor(
                                    g[j][:, sl_g], ps[:], b32[j][:, sl_g], Alu.add
                                )
                                if not last:
                                    nc.scalar.activation(
                                        nxt[j][:, sl_t], g[j][:, sl_g], Act.Square,
                                        bias=negh[:], scale=1.0,
                                    )
                                else:
                                    nc.vector.tensor_scalar(
                                        ob[j][:, sl_t], g[j][:, sl_g],
                                        1e-4, 1.0 - 1e-4, Alu.max, Alu.min,
                                    )
                    cur, nxt = nxt, cur

                for j in range(2):
                    nc.gpsimd.dma_start(
                        outT[j * 128:(j + 1) * 128, col0:col0 + CHUNK], ob[j][:]
                    )
    nc.compile()
    return nc


def _get_nc():
    global _CACHED_NC
    if _CACHED_NC is None:
        _CACHED_NC = _build_nc()
    return _CACHED_NC


def _fold_constants(r, eps, beta, K_local, W_cc):
    """Host-side fold of the per-step linear operator into A_neg / qc."""
    pad = KTAPS // 2
    cp = np.arange(C)[:, None]
    c = np.arange(C)[None, :]
    j = (cp - c + pad) % C
    B = np.where(j < KTAPS, K_local.astype(np.float64)[np.minimum(j, KTAPS - 1)], 0.0)
    A = (1.0 - beta.astype(np.float64))[None, :] * (
        (1.0 - eps.astype(np.float64))[None, :] * np.eye(C)
        + eps.astype(np.float64)[None, :] * 0.5 * (B + W_cc.astype(np.float64))
    )
    A_r = r.astype(np.float64)[:, None] * A
    A_neg = (-A_r).astype(np.float32)          # [C, C]; g' = t @ A_neg + bias2
    qc = (0.25 * A_r.sum(axis=0)).astype(np.float32)   # [C]
    return A_neg, qc


def _pack_inputs(drive, r, eps, beta, K_local, W_cc):
    A_neg, qc = _fold_constants(r, eps, beta, K_local, W_cc)
    # lhsT blocks: block (k, m) at columns (k*2+m)*128; identity at 512:640
    blocks = [A_neg[k * 128:(k + 1) * 128, m * 128:(m + 1) * 128]
              for k in range(2) for m in range(2)]
    blocks.append(np.eye(128, dtype=np.float32))
    a_blk = np.concatenate(blocks, axis=1).astype(np.float32)   # [128, 640]
    vecs = np.stack(
        [beta[0:128], beta[128:256], qc[0:128], qc[128:256]], axis=1
    ).astype(np.float32)                       # [128, 4]
    driveT = np.ascontiguousarray(drive.T.astype(np.float32))   # [C, N]
    in_maps = []
    for i in range(N_CORES):
        shard = np.ascontiguousarray(driveT[:, i * N_SHARD:(i + 1) * N_SHARD])
        in_maps.append({"driveT": shard, "a_blk": a_blk, "vecs": vecs})
    return in_maps


def run(drive, r, eps, beta, K_local, W_cc, trace=False, trace_kwargs=None):
    from concourse.bass_utils import run_bass_kernel_spmd

    nc = _get_nc()
    in_maps = _pack_inputs(drive, r, eps, beta, K_local, W_cc)
    res = run_bass_kernel_spmd(
        nc, in_maps, core_ids=list(range(N_CORES)),
        trace=trace, **(trace_kwargs or {}),
    )
    outT = np.concatenate([res.results[i]["outT"] for i in range(N_CORES)], axis=1)
    out = np.ascontiguousarray(outT.T).astype(np.float32)
    return out, res


def kernel(drive, r, eps, beta, K_local, W_cc):
    out, _ = run(
        np.asarray(drive), np.asarray(r), np.asarray(eps), np.asarray(beta),
        np.asarray(K_local), np.asarray(W_cc),
    )
    return out
